# revision 42
# baseline (speedup 1.0000x reference)
"""AdptWeightBCEDiceLoss Trainium2 kernel.

Full inputs y_pred/y_target [32,1,512,512] f32 -> scalar f32 loss.

Strategy (pure data-parallel over 8 NeuronCores, 4 images each):
  weight = 1 + 5|avgpool31(t) - t|.  The 31x31 box filter is separable and
  BOTH passes run as banded-0/1-matrix matmuls on TensorE: pass 1 pools
  along w using a host-transposed copy of the target (tb5T); the single
  intermediate transpose rides the DMA xbar (4 descriptors per image);
  pass 2 pools along h and lands the centered box sum in natural
  orientation directly in PSUM, where the VectorE merge reads it.
  softplus/sigmoid stay inside two ACT table sets: F = sigmoid(-x)
  (sigmoid set), softplus = -ln(F) (natural_log set, phase-batched last).
  All spatial reductions ride accum_out on the producing instruction (ACT
  activations + the HW-validated custom affine_mul_reduce DVE op -- the
  ISA TensorScalarPtr/TensorTensorReduce accum variants fault TRN2).
  Each core ships a [128, 6*n_img] tile of per-partition partial sums;
  the host does the final scalar math in float64.

Per image i (N = 512*512), with q = 5|avgpool - t|, F = 1 - sigmoid(x):
  acc columns: 0: sum q               -> A = N + sq   (= sum weight)
               1: sum (1+q)*5t        -> su5
               2: sum (1+q)*F         -> sv
               3: sum (1+q)*5t*F      -> sx5
               4: 5*sum x*t           -> sz5
               5: sum ln F            -> slnF  (= -sum softplus)
  B = (su5 - sx5)/5,  C = (A - sv) + su5/5,  G = -slnF - sz5/5.
"""

import numpy as np

import concourse.bacc as bacc
import concourse.bass as bass
import concourse.tile as tile
from concourse import mybir
from concourse.bass_utils import run_bass_kernel_spmd

F32 = mybir.dt.float32
BF16 = mybir.dt.bfloat16

H = W = 512
RB = 4              # 512 rows / 128 partitions
KPOOL = 31
PADB = 15
NPIX = H * W
N_CORES = 8
IMG_PER_CORE = 4
SMOOTH = 1e-8


def band_matrix_blocks() -> np.ndarray:
    """B[h_in, h_out] = 1 iff |h_in - h_out| <= 15, laid out as
    [128, (ri*4+ro)*128 + m] so bb[:, pair*128:(pair+1)*128] is the
    stationary [K=128, M=128] block for input row-block ri, output ro."""
    import ml_dtypes

    idx = np.arange(H)
    bm = (np.abs(idx[:, None] - idx[None, :]) <= PADB).astype(ml_dtypes.bfloat16)
    return np.ascontiguousarray(
        bm.reshape(RB, 128, RB, 128).transpose(1, 0, 2, 3).reshape(128, RB * RB * 128)
    )


def build_nc(n_img: int = IMG_PER_CORE) -> bacc.Bacc:
    nc = bacc.Bacc("TRN2", target_bir_lowering=False, debug=False)
    pb_d = nc.dram_tensor("pb", [n_img, H, W], BF16, kind="ExternalInput")
    tb5_d = nc.dram_tensor("tb5", [n_img, H, W], BF16, kind="ExternalInput")
    tb5t_d = nc.dram_tensor("tb5T", [n_img, H, W], BF16, kind="ExternalInput")
    bb_d = nc.dram_tensor("bband", [128, RB * RB * 128], BF16, kind="ExternalInput")
    acc_d = nc.dram_tensor("acc", [128, 6 * n_img], F32, kind="ExternalOutput")

    with tile.TileContext(nc) as tc:
        _body(tc, pb_d, tb5_d, tb5t_d, bb_d, acc_d, n_img)
    nc.compile()
    return nc


def _band_pool(nc, bb, psum, moving3):
    """psum[:, ro, :] = sum_ri B[ri,ro]^T @ moving3[:, ri, :] (band: only
    |ri - ro| <= 1 blocks are nonzero)."""
    for ro in range(RB):
        ris = [r for r in (ro - 1, ro, ro + 1) if 0 <= r < RB]
        for k, ri in enumerate(ris):
            pair = ri * RB + ro
            nc.tensor.matmul(
                psum[:, ro, :],
                bb[:, pair * 128:(pair + 1) * 128],
                moving3[:, ri, :],
                start=(k == 0),
                stop=(k == len(ris) - 1),
            )


def _body(tc, pb_d, tb5_d, tb5t_d, bb_d, acc_d, n_img):
    nc = tc.nc
    SUB = mybir.AluOpType.subtract
    ACTF = mybir.ActivationFunctionType
    QSCALE = float(1.0 / (KPOOL * KPOOL))

    with (
        tc.tile_pool(name="const", bufs=1) as constp,
        tc.tile_pool(name="pb", bufs=2) as pbp,
        tc.tile_pool(name="tb5", bufs=2) as tb5p,
        tc.tile_pool(name="tb5t", bufs=2) as tb5tp,
        tc.tile_pool(name="s1e", bufs=2) as s1ep,
        tc.tile_pool(name="s1et", bufs=2) as s1etp,
        tc.tile_pool(name="dmrg", bufs=2) as dp,
        tc.tile_pool(name="qt", bufs=2) as qp,
        tc.tile_pool(name="st", bufs=2) as sp_,
        tc.tile_pool(name="ft", bufs=n_img) as fp_,
        tc.tile_pool(name="ut", bufs=2) as up,
        tc.tile_pool(name="vscr", bufs=2) as vp,
        tc.tile_pool(name="xscr", bufs=2) as xp,
        tc.tile_pool(name="zscr", bufs=2) as zp,
        tc.tile_pool(name="ps1", bufs=1, space=bass.MemorySpace.PSUM) as ps1p,
        tc.tile_pool(name="ps2", bufs=1, space=bass.MemorySpace.PSUM) as ps2p,
    ):
        bb = constp.tile([128, RB * RB * 128], BF16)
        nc.sync.dma_start(bb[:], bb_d.ap()[:, :])
        acc = constp.tile([128, 6 * n_img], F32)
        zb = constp.tile([128, 1], F32)
        nc.vector.memset(zb[:], 0.0)

        f_tiles = []
        for i in range(n_img):
            c = 6 * i
            # host-precast inputs: PB = bf16(pred), TB5 = bf16(5t),
            # TB5T = bf16(5t)^T (pass-1 moving operand)
            TB5T = tb5tp.tile([128, RB, H], BF16)
            nc.sync.dma_start(TB5T[:], tb5t_d.ap()[i].rearrange("(rb p) w -> p rb w", p=128))
            TB5t = tb5p.tile([128, RB, W], BF16)
            nc.sync.dma_start(TB5t[:], tb5_d.ap()[i].rearrange("(rb p) w -> p rb w", p=128))
            PB = pbp.tile([128, RB, W], BF16)
            nc.sync.dma_start(PB[:], pb_d.ap()[i].rearrange("(rb p) w -> p rb w", p=128))
            Pf = PB[:].rearrange("p rb w -> p (rb w)")
            TB5 = TB5t[:].rearrange("p rb w -> p (rb w)")

            # ---- pass 1: pool along w on the transposed image
            ps1 = ps1p.tile([128, RB, H], F32)
            _band_pool(nc, bb, ps1, TB5T[:, :, :])

            # ---- evacuate scaled by 1/961 -> s1e = (5/961) w-boxsum ^T
            s1e = s1ep.tile([128, RB, H], BF16)
            nc.scalar.activation(s1e[:], ps1[:], ACTF.Copy, scale=QSCALE)

            # ---- transpose back to natural via DMA xbar
            s1et = s1etp.tile([128, RB, W], BF16)
            for rb in range(RB):
                nc.sync.dma_start_transpose(
                    out=s1et[:, :, 128 * rb:128 * rb + 128], in_=s1e[:, rb, :]
                )

            # ---- pass 2: pool along h -> 5*avgpool, natural, in PSUM
            ps2 = ps2p.tile([128, RB, W], F32)
            _band_pool(nc, bb, ps2, s1et[:, :, :])

            # ---- D = 5t - 5*avgpool (sign eaten by abs), straight off PSUM
            D = dp.tile([128, RB, W], BF16)
            nc.vector.tensor_tensor(D[:], TB5t[:], ps2[:], SUB)
            Df = D[:].rearrange("p rb w -> p (rb w)")

            # ---- q = |D| = 5|avgpool - t| on ACT; accum -> sum q
            q = qp.tile([128, RB * W], BF16)
            nc.scalar.activation(
                q[:], Df, ACTF.Abs, bias=zb[:], accum_out=acc[:, c + 0:c + 1]
            )

            # ---- F = sigmoid(-x) = 1 - p;  softplus(x) = -ln(F) (phase 2)
            F = fp_.tile([128, RB * W], BF16)
            nc.scalar.activation(F[:], Pf, ACTF.Sigmoid, bias=zb[:], scale=-1.0)
            f_tiles.append(F)

            # ---- products via the custom affine_mul_reduce DVE op
            u = up.tile([128, RB * W], BF16)
            nc.vector.affine_mul_reduce(
                u[:], acc[:, c + 1:c + 2], q[:], TB5, 1.0, 1.0
            )
            vscr = vp.tile([128, RB * W], BF16)
            nc.vector.affine_mul_reduce(
                vscr[:], acc[:, c + 2:c + 3], q[:], F[:], 1.0, 1.0
            )
            xscr = xp.tile([128, RB * W], BF16)
            nc.vector.affine_mul_reduce(
                xscr[:], acc[:, c + 3:c + 4], u[:], F[:], 1.0, 0.0
            )
            zscr = zp.tile([128, RB * W], BF16)
            nc.vector.affine_mul_reduce(
                zscr[:], acc[:, c + 4:c + 5], Pf, TB5, 1.0, 0.0
            )

        # ---- phase 2: sum softplus = -sum ln(F), batched so the ACT
        # natural_log table loads exactly once. The Ln bias tile depends on
        # the last F so the scheduler cannot interleave Ln's (natural_log
        # set) between Sigmoids (sigmoid set).
        zb2 = constp.tile([128, 1], F32)
        nc.vector.tensor_scalar_mul(zb2[:], f_tiles[-1][:, 0:1], 0.0)
        for i in range(n_img):
            lnscr = sp_.tile([128, RB * W], BF16)
            nc.scalar.activation(
                lnscr[:], f_tiles[i][:], ACTF.Ln, bias=zb2[:],
                accum_out=acc[:, 6 * i + 5:6 * i + 6],
            )

        nc.sync.dma_start(acc_d.ap()[:, :], acc[:])


def combine(acc_list, n_img_total):
    """acc_list: list of [128, 6*n_img] per-core arrays -> scalar loss."""
    a = np.concatenate(
        [a.reshape(128, -1, 6) for a in acc_list], axis=1
    ).astype(np.float64)          # [128, n_img_total, 6]
    s = a.sum(axis=0)             # [n_img_total, 6]: q,u5,v,x5,z5,lnF
    sq, su5, sv, sx5, sz5, slnF = (s[:, j] for j in range(6))
    A = NPIX + sq
    B = (su5 - sx5) / 5.0
    C = (A - sv) + su5 / 5.0
    G = -slnF - sz5 / 5.0
    bce = G.sum() / (n_img_total * NPIX)
    w_bce = (A * bce + SMOOTH) / (A + SMOOTH)
    w_iou = 1.0 - (B + 1.0 + SMOOTH) / (C - B + 1.0 + SMOOTH)
    return np.float32(np.mean(w_bce + w_iou))


def host_inputs(pred, targ):
    """pred/targ [n,512,512] f32 -> (pb, tb5, tb5T) bf16 host-side."""
    import ml_dtypes

    pb = np.ascontiguousarray(pred.astype(ml_dtypes.bfloat16))
    t5 = (5.0 * targ).astype(ml_dtypes.bfloat16)
    tb5 = np.ascontiguousarray(t5)
    tb5T = np.ascontiguousarray(t5.transpose(0, 2, 1))
    return pb, tb5, tb5T


def kernel(y_pred: np.ndarray, y_target: np.ndarray) -> np.ndarray:
    pred = np.ascontiguousarray(np.asarray(y_pred, dtype=np.float32).reshape(-1, H, W))
    targ = np.ascontiguousarray(np.asarray(y_target, dtype=np.float32).reshape(-1, H, W))
    n_total = pred.shape[0]
    assert n_total == N_CORES * IMG_PER_CORE

    nc = build_nc(IMG_PER_CORE)
    bb = band_matrix_blocks()
    pb, tb5, tb5T = host_inputs(pred, targ)
    in_maps = [
        {
            "pb": pb[c * IMG_PER_CORE:(c + 1) * IMG_PER_CORE],
            "tb5": tb5[c * IMG_PER_CORE:(c + 1) * IMG_PER_CORE],
            "tb5T": tb5T[c * IMG_PER_CORE:(c + 1) * IMG_PER_CORE],
            "bband": bb,
        }
        for c in range(N_CORES)
    ]
    res = run_bass_kernel_spmd(nc, in_maps, list(range(N_CORES)))
    accs = [res.results[c]["acc"] for c in range(N_CORES)]
    return np.asarray(combine(accs, n_total))
